# revision 7
# baseline (speedup 1.0000x reference)
"""LightGCN-style CFGCN forward (propagate + BPR-ish loss) on 8 Trainium2 cores.

Strategy (dst-partitioned 1D graph sharding):
  - Nodes sharded 25000/core; per layer the pre-scaled embedding table
    (bf16, rows padded to 256B) is AllGathered into every core's HBM.
  - Each core gathers its in-edges' source rows with the SWDGE dma_gather
    primitive (int16 shard-local indices, edges bucketed by
    (dst-block, src-shard) and padded to 128-slot tiles).
  - Segment-sum runs on TensorE: per 128-edge tile a one-hot selection
    matrix S (built on VectorE via iota==dstrel) is matmul'd with the
    gathered messages, accumulating per-dst-block in PSUM.
  - PSUM is evicted with fused *sqrt_deg scaling into the f32 accumulator
    and the next layer's bf16 table staging.
  - Scoring (u/p/n dot products, softplus, ego L2 reg) is data-parallel
    over the 16384 batch (2048/core) + a final AllReduce.
"""

import sys
import numpy as np

sys.path.insert(0, "/opt/trn_rl_repo")

import ml_dtypes
import concourse.bacc as bacc
import concourse.bass as bass
import concourse.mybir as mybir
from concourse.bass_utils import run_bass_kernel_spmd
from concourse.tile import TileContext

# ---------------------------------------------------------------- config
N_NODES = 200000
N_EDGES = 1250000
D = 64
DC = 128            # table row padded to 128 cols bf16 = 256B
N_CORES = 8
BATCH = 16384
N_LAYERS = 3
LAM = 0.001
PART = N_NODES // N_CORES            # 25000
NB = (PART + 127) // 128             # 196 dst blocks / core
TROWS = NB * 128 + 16                # padded shard table rows (+16 zeroed pad)
GSIZE = 49                           # dst blocks per group (<=56: 7 PSUM banks)
NGROUP = (NB + GSIZE - 1) // GSIZE
BPC = BATCH // N_CORES               # 2048 batch items / core
BCOLS = BPC // 128                   # 16
F32 = mybir.dt.float32
BF16 = mybir.dt.bfloat16
I16 = mybir.dt.int16
I32 = mybir.dt.int32
AX = mybir.AxisListType
OP = mybir.AluOpType
AF = mybir.ActivationFunctionType

_MAX_WAIT_SPLIT_TYPES = ("InstDrain", "InstEventSemaphore", "InstHalt", "InstNoOp")


def _split_ctrl_waits(nc, max_waits=1):
    """walrus in this container accepts only one sync-wait on TPB_CTRL
    encodings; move extra waits onto single-wait nop carriers."""
    for fn in nc.m.functions:
        for blk in fn.blocks:
            insts = list(blk.instructions)
            out, changed = [], False
            for ins in insts:
                si = ins.sync_info
                nw = len(si.on_wait) if si and si.on_wait else 0
                if nw > max_waits and type(ins).__name__ in _MAX_WAIT_SPLIT_TYPES:
                    waits = list(si.on_wait)
                    for j, w in enumerate(waits[max_waits:]):
                        nop = mybir.InstNoOp(name=f"{ins.name}_ws{j}", ins=[], outs=[])
                        nop.engine = ins.engine
                        nop.sync_info = mybir.SyncInfo(on_wait=[w], on_update=[])
                        out.append(nop)
                    si.on_wait = waits[:max_waits]
                    changed = True
                out.append(ins)
            if changed:
                blk.instructions = out
    return nc


# ---------------------------------------------------------------- host prep
def _wrap_idx(arr):
    """[n] -> [128, n//16] int16 wrapped in 16 partitions, replicated 8x."""
    n = len(arr)
    w = arr.astype(np.int16).reshape(n // 16, 16).T
    return np.tile(w, (8, 1))


def _preprocess(emb, sqrt_deg, src, dst, users, pos, neg, n_users):
    src = np.asarray(src).astype(np.int64)
    dst = np.asarray(dst).astype(np.int64)
    emb = np.asarray(emb, dtype=np.float32)
    sd = np.asarray(sqrt_deg, dtype=np.float32)

    core = dst // PART
    dloc = dst - core * PART
    b = dloc // 128
    ps = src // PART
    sloc = (src - ps * PART).astype(np.int32)
    drel = (dloc - b * 128).astype(np.float32)

    bucket = (core * NB + b) * N_CORES + ps          # [E]
    order = np.argsort(bucket, kind="stable")
    counts = np.bincount(bucket, minlength=N_CORES * NB * N_CORES)
    counts3 = counts.reshape(N_CORES, NB, N_CORES)   # [core, b, ps]
    tmax = (np.max(counts3, axis=0) + 127) // 128    # [b, ps] common tiles
    tmax = np.maximum(tmax, 1)

    # common slot layout: g-major, ps, then blocks of g
    slot_off = np.zeros((NB, N_CORES), np.int64)     # slot offset of bucket
    call_meta = []                                   # (g, ps, n_idx, off)
    cur = 0
    tile_blocks = []                                 # per global tile: (g, ps, j)
    for g in range(NGROUP):
        blo, bhi = g * GSIZE, min((g + 1) * GSIZE, NB)
        for p in range(N_CORES):
            off0 = cur
            for bb in range(blo, bhi):
                slot_off[bb, p] = cur
                cur += int(tmax[bb, p]) * 128
                for _ in range(int(tmax[bb, p])):
                    tile_blocks.append((g, p, bb))
            call_meta.append((g, p, cur - off0, off0))
    tot_slots = cur
    assert tot_slots % 128 == 0
    t_total = tot_slots // 128
    for _, _, n, _ in call_meta:
        assert n <= 12800, f"gather call too big: {n}"

    # per-core streams
    starts = np.zeros(N_CORES * NB * N_CORES + 1, np.int64)
    np.cumsum(counts, out=starts[1:])
    skey = bucket[order]
    rank = np.arange(N_EDGES) - starts[skey]
    pos_in_stream = slot_off[(skey // N_CORES) % NB, skey % N_CORES] + rank
    core_sorted = skey // (NB * N_CORES)

    idx_streams = np.full((N_CORES, tot_slots), PART, np.int32)
    drel_streams = np.full((N_CORES, tot_slots), -1.0, np.float32)
    idx_streams[core_sorted, pos_in_stream] = sloc[order]
    drel_streams[core_sorted, pos_in_stream] = drel[order]

    # device tensors per core
    per_core = []
    sd2 = sd * sd
    users = np.asarray(users).astype(np.int64)
    posn = np.asarray(pos).astype(np.int64) + int(n_users)
    negn = np.asarray(neg).astype(np.int64) + int(n_users)

    def trow(node):
        return ((node // PART) * TROWS + node % PART).astype(np.int32)

    def bcol(node32):  # [2048] -> [128, 16] slot (p, j) = elem j*128+p
        return node32.reshape(BCOLS, 128).T.copy()

    iota = np.tile(np.arange(128, dtype=np.float32), (128, 1)).astype(ml_dtypes.bfloat16)
    ones = np.ones((128, 1), np.float32)

    for c in range(N_CORES):
        lo = c * PART
        e = emb[lo:lo + PART]                        # [25000, 64]
        epb = np.zeros((128, NB * D), np.float32)
        ex = np.zeros((NB * 128, D), np.float32)
        ex[:PART] = e
        epb[:] = ex.reshape(NB, 128, D).transpose(1, 0, 2).reshape(128, NB * D)
        sdx = np.zeros(NB * 128, np.float32)
        sdx[:PART] = sd[lo:lo + PART]
        sd_pb = sdx.reshape(NB, 128).T.copy()
        sd2x = np.zeros(NB * 128, np.float32)
        sd2x[:PART] = sd2[lo:lo + PART]
        sd2_pb = sd2x.reshape(NB, 128).T.copy()

        gidx = np.hstack([_wrap_idx(idx_streams[c, off:off + n])
                          for (_, _, n, off) in call_meta])
        drel_t = drel_streams[c].reshape(t_total, 128).T.copy()

        cnt = np.bincount(users, minlength=N_NODES).astype(np.float32)
        cnt += np.bincount(posn, minlength=N_NODES)
        cnt += np.bincount(negn, minlength=N_NODES)
        cx = np.zeros(NB * 128, np.float32)
        cx[:PART] = cnt[lo:lo + PART]
        cnt_pb = cx.reshape(NB, 128).T.copy()

        sl = slice(c * BPC, (c + 1) * BPC)
        per_core.append({
            "cnt_pb": cnt_pb,
            "emb_pb": epb,
            "sd_pb": sd_pb,
            "sd2_pb": sd2_pb,
            "gidx": gidx,
            "dstrel": drel_t,
            "iota": iota,
            "ones": ones,
            "u_tr": bcol(trow(users[sl])), "p_tr": bcol(trow(posn[sl])),
            "n_tr": bcol(trow(negn[sl])),
        })

    meta = {
        "call_meta": call_meta,
        "tile_blocks": tile_blocks,
        "t_total": t_total,
        "gidx16": per_core[0]["gidx"].shape[1],
    }
    return per_core, meta


# ---------------------------------------------------------------- builder
def _build(meta):
    call_meta = meta["call_meta"]
    tile_blocks = meta["tile_blocks"]
    t_total = meta["t_total"]
    gidx16 = meta["gidx16"]

    # per-block ordered tile list (global tile ids)
    blk_tiles = [[] for _ in range(NB)]
    for t, (g, p, bb) in enumerate(tile_blocks):
        blk_tiles[bb].append(t)
    # chunk (g,ps) -> (tile range, idx16 range)
    chunk_of = {}
    t0 = 0
    off16 = 0
    for (g, p, n, off) in call_meta:
        nt = n // 128
        chunk_of[(g, p)] = (t0, nt, off16, n)
        t0 += nt
        off16 += n // 16
    maxt = max(nt for (_, nt, _, _) in chunk_of.values())

    nc = bacc.Bacc("TRN2")
    cnt_pb = nc.dram_tensor("cnt_pb", [128, NB], F32, kind="ExternalInput")
    emb_pb = nc.dram_tensor("emb_pb", [128, NB * D], F32, kind="ExternalInput")
    sd_pb = nc.dram_tensor("sd_pb", [128, NB], F32, kind="ExternalInput")
    sd2_pb = nc.dram_tensor("sd2_pb", [128, NB], F32, kind="ExternalInput")
    gidx = nc.dram_tensor("gidx", [128, gidx16], I16, kind="ExternalInput")
    dstrel = nc.dram_tensor("dstrel", [128, t_total], F32, kind="ExternalInput")
    iota = nc.dram_tensor("iota", [128, 128], BF16, kind="ExternalInput")
    ones = nc.dram_tensor("ones", [128, 1], F32, kind="ExternalInput")
    btens = {nm: nc.dram_tensor(nm, [128, BCOLS], I32, kind="ExternalInput")
             for nm in ("u_tr", "p_tr", "n_tr")}
    loss_o = nc.dram_tensor("loss", [1, 2], F32, kind="ExternalOutput")

    bounce = nc.dram_tensor("bounce", [TROWS, DC], BF16)
    table = nc.dram_tensor("table", [N_CORES * TROWS, DC], BF16)
    red_i = nc.dram_tensor("red_i", [1, 2], F32)
    red_o = nc.dram_tensor("red_o", [1, 2], F32)

    groups = [(g * GSIZE, min((g + 1) * GSIZE, NB)) for g in range(NGROUP)]
    rg = [list(range(N_CORES))]

    with TileContext(nc) as tc:
        with (tc.tile_pool(name="per", bufs=1) as per,
              tc.tile_pool(name="ring", bufs=2) as ring,
              tc.tile_pool(name="psr", bufs=1, space="PSUM") as psr):
            # resident tiles
            acc = per.tile([128, NB * D], F32, name="acc")
            sd_t = per.tile([128, NB], F32, name="sd_t")
            sd2_t = per.tile([128, NB], F32, name="sd2_t")
            gidx_t = per.tile([128, gidx16], I16, name="gidx_t")
            drel_t = per.tile([128, t_total], F32, name="drel_t")
            iota_t = per.tile([128, 128], BF16, name="iota_t")
            ones_t = per.tile([128, 1], F32, name="ones_t")
            nc.sync.dma_start(acc[:], emb_pb[:])
            nc.sync.dma_start(sd_t[:], sd_pb[:])
            nc.sync.dma_start(sd2_t[:], sd2_pb[:])
            nc.sync.dma_start(gidx_t[:], gidx[:])
            nc.sync.dma_start(drel_t[:], dstrel[:])
            nc.sync.dma_start(iota_t[:], iota[:])
            nc.sync.dma_start(ones_t[:], ones[:])
            zpad = per.tile([16, DC], BF16, name="zpad")
            nc.vector.memset(zpad[:], 0.0)
            nc.sync.dma_start(bounce[NB * 128:TROWS, :], zpad[:])
            cnt_t = per.tile([128, NB], F32, name="cnt_t")
            nc.sync.dma_start(cnt_t[:], cnt_pb[:])
            regcol = per.tile([128, NB, 1], F32, name="regcol")
            regp = per.tile([128, 1], F32, name="regp")
            for g0 in range(0, NB, GSIZE):
                g1 = min(g0 + GSIZE, NB)
                rsq = ring.tile([128, GSIZE * D], F32, tag="rsq", name=f"rsq_{g0}")
                nc.vector.tensor_tensor(
                    out=rsq[:, 0:(g1 - g0) * D],
                    in0=acc[:, g0 * D:g1 * D], in1=acc[:, g0 * D:g1 * D],
                    op=OP.mult)
                nc.vector.reduce_sum(
                    regcol[:, g0:g1, :],
                    rsq[:, 0:(g1 - g0) * D].rearrange("p (b d) -> p b d", d=D),
                    axis=AX.X)
            nc.vector.tensor_tensor(
                out=regcol[:, :, 0], in0=regcol[:, :, 0], in1=cnt_t[:],
                op=OP.mult)
            nc.vector.reduce_sum(regp[:], regcol[:], axis=AX.XY)

            def stage_group(g, blo, bhi, src_fn, li):
                """write bf16 staged table rows for group's blocks, DMA out."""
                n = bhi - blo
                st = ring.tile([128, GSIZE * DC], BF16, tag="stage",
                               name=f"st_{li}_{g}")
                for j in range(n):
                    src_fn(j, st[:, j * DC: j * DC + D])
                dstv = bounce[blo * 128:(blo + n) * 128, :].rearrange(
                    "(j p) d -> p j d", p=128)[:, :, 0:D]
                nc.sync.dma_start(dstv, st[:, 0:n * DC].rearrange(
                    "p (j d) -> p j d", d=DC)[:, :, 0:D])

            # ---- initial table: emb * sd
            for g, (blo, bhi) in enumerate(groups):
                def src0(j, out, blo=blo):
                    bb = blo + j
                    nc.vector.tensor_scalar(
                        out=out, in0=acc[:, bb * D:(bb + 1) * D],
                        scalar1=sd_t[:, bb:bb + 1], scalar2=None, op0=OP.mult)
                stage_group(g, blo, bhi, src0, "i")
            nc.gpsimd.collective_compute(
                "AllGather", OP.bypass, replica_groups=rg,
                ins=[bounce[:].opt()], outs=[table[:].opt()])

            # ---- layers
            for li in range(N_LAYERS):
                last = li == N_LAYERS - 1
                for g, (blo, bhi) in enumerate(groups):
                    nblk = bhi - blo
                    nbank = (nblk + 7) // 8
                    banks = [psr.tile([128, 512], F32, tag=f"pb{k}",
                                      name=f"pb_{li}_{g}_{k}")
                             for k in range(nbank)]

                    def pslice(bb):
                        j = bb - blo
                        return banks[j // 8][:, (j % 8) * 64:(j % 8) * 64 + 64]

                    chunks = []
                    for p in range(N_CORES):
                        ct0, nt, o16, n = chunk_of[(g, p)]
                        m = ring.tile([128, maxt * DC], BF16, tag="msgs",
                                      name=f"m_{li}_{g}_{p}")
                        s = ring.tile([128, maxt * 128], BF16, tag="sgen",
                                      name=f"s_{li}_{g}_{p}")
                        nc.gpsimd.dma_gather(
                            m[:, 0:nt * DC].rearrange("p (t d) -> p t d", d=DC),
                            table[p * TROWS:(p + 1) * TROWS, :],
                            gidx_t[:, o16:o16 + n // 16],
                            n, n, DC, single_packet=False)
                        chunks.append((ct0, nt, m, s))
                    for p in range(N_CORES):
                        ct0, nt, m, s = chunks[p]
                        for tl in range(nt):
                            t = ct0 + tl
                            bb = tile_blocks[t][2]
                            sv = s[:, tl * 128:(tl + 1) * 128]
                            nc.vector.tensor_scalar(
                                out=sv, in0=iota_t[:],
                                scalar1=drel_t[:, t:t + 1], scalar2=None,
                                op0=OP.is_equal)
                            tlist = blk_tiles[bb]
                            nc.tensor.matmul(
                                pslice(bb), lhsT=sv,
                                rhs=m[:, tl * DC:tl * DC + D],
                                start=(t == tlist[0]), stop=(t == tlist[-1]))
                    # evictions
                    for bb in range(blo, bhi):
                        tmp = ring.tile([128, 64], F32, tag="ev",
                                        name=f"ev_{li}_{g}_{bb}")
                        nc.vector.tensor_scalar(
                            out=tmp[:], in0=pslice(bb),
                            scalar1=sd_t[:, bb:bb + 1], scalar2=None,
                            op0=OP.mult)
                        nc.vector.tensor_tensor(
                            out=acc[:, bb * D:(bb + 1) * D],
                            in0=acc[:, bb * D:(bb + 1) * D], in1=tmp[:],
                            op=OP.add)
                    if not last:
                        def srcn(j, out, blo=blo):
                            bb = blo + j
                            nc.vector.tensor_scalar(
                                out=out, in0=pslice(bb),
                                scalar1=sd2_t[:, bb:bb + 1], scalar2=None,
                                op0=OP.mult)
                        stage_group(g, blo, bhi, srcn, li)
                if not last:
                    nc.gpsimd.collective_compute(
                        "AllGather", OP.bypass, replica_groups=rg,
                        ins=[bounce[:].opt()], outs=[table[:].opt()])

            # ---- prop table: acc * 0.25
            for g, (blo, bhi) in enumerate(groups):
                def srcp(j, out, blo=blo):
                    bb = blo + j
                    nc.vector.tensor_scalar_mul(
                        out, acc[:, bb * D:(bb + 1) * D], 0.25)
                stage_group(g, blo, bhi, srcp, "p")
            nc.gpsimd.collective_compute(
                "AllGather", OP.bypass, replica_groups=rg,
                ins=[bounce[:].opt()], outs=[table[:].opt()])

            # ---- scoring
            bt = {nm: per.tile([128, BCOLS], I32, name=f"bt_{nm}")
                  for nm in btens}
            for nm in btens:
                nc.sync.dma_start(bt[nm][:], btens[nm][:])
            prop = {}
            for nm in ("u_tr", "p_tr", "n_tr"):
                pt = per.tile([128, BCOLS, DC], BF16, name=f"prop_{nm}")
                for j in range(BCOLS):
                    nc.gpsimd.indirect_dma_start(
                        out=pt[:, j, :], out_offset=None, in_=table[:],
                        in_offset=bass.IndirectOffsetOnAxis(
                            ap=bt[nm][:, j:j + 1], axis=0))
                prop[nm] = pt
            def dots(a, b, name):
                m = per.tile([128, BCOLS, D], F32, name=f"dm_{name}")
                nc.vector.tensor_tensor(out=m[:], in0=a[:, :, 0:D],
                                        in1=b[:, :, 0:D], op=OP.mult)
                r = per.tile([128, BCOLS, 1], F32, name=f"dr_{name}")
                nc.vector.reduce_sum(r[:], m[:], axis=AX.X)
                return r

            ps_s = dots(prop["u_tr"], prop["p_tr"], "pos")
            ns_s = dots(prop["u_tr"], prop["n_tr"], "neg")
            diff = per.tile([128, BCOLS, 1], F32, name="diff")
            nc.vector.tensor_tensor(out=diff[:], in0=ns_s[:], in1=ps_s[:],
                                    op=OP.subtract)
            expd = per.tile([128, BCOLS, 1], F32, name="expd")
            nc.scalar.activation(expd[:], diff[:], AF.Exp)
            sp = per.tile([128, BCOLS, 1], F32, name="sp")
            nc.scalar.activation(sp[:], expd[:], AF.Ln, bias=1.0)

            two = per.tile([128, 2], F32, name="two")
            nc.vector.reduce_sum(two[:, 0:1], sp[:], axis=AX.XY)
            nc.vector.tensor_copy(two[:, 1:2], regp[:])
            rp = psr.tile([128, 2], F32, tag="rps", name="rps")
            nc.tensor.matmul(rp[0:1, 0:2], lhsT=ones_t[:], rhs=two[:],
                             start=True, stop=True)
            redt = per.tile([1, 2], F32, name="redt")
            nc.vector.tensor_copy(redt[:], rp[0:1, 0:2])
            nc.sync.dma_start(red_i[:], redt[:])
            nc.gpsimd.collective_compute(
                "AllReduce", OP.add, replica_groups=rg,
                ins=[red_i[:].opt()], outs=[red_o[:].opt()])
            fin = per.tile([1, 2], F32, name="fin")
            nc.sync.dma_start(fin[:], red_o[:])
            nc.sync.dma_start(loss_o[:], fin[:])
    nc.compile()
    _split_ctrl_waits(nc)
    return nc


def kernel(**inputs) -> np.ndarray:
    per_core, meta = _preprocess(
        inputs["emb"], inputs["sqrt_deg"], inputs["src"], inputs["dst"],
        inputs["users"], inputs["pos"], inputs["neg"], inputs["n_users"])
    nc = _build(meta)
    res = None
    for attempt in range(3):
        try:
            res = run_bass_kernel_spmd(nc, per_core, core_ids=list(range(N_CORES)))
            break
        except Exception:
            # transient NRT exec-unit flake: device recovers on re-dispatch
            if attempt == 2:
                raise
            import time as _t
            _t.sleep(5)
    two = res.results[0]["loss"].reshape(2).astype(np.float64)
    loss = two[0] / BATCH + LAM * 0.5 * two[1] / BATCH
    return np.asarray(loss, dtype=np.float32).reshape(())


# revision 8
# speedup vs baseline: 1.0703x; 1.0703x over previous
"""LightGCN-style CFGCN forward (propagate + BPR-ish loss) on 8 Trainium2 cores.

Strategy (dst-partitioned 1D graph sharding):
  - Nodes sharded 25000/core; per layer the pre-scaled embedding table
    (bf16, rows padded to 256B) is AllGathered into every core's HBM.
  - Each core gathers its in-edges' source rows with the SWDGE dma_gather
    primitive (int16 shard-local indices, edges bucketed by
    (dst-block, src-shard) and padded to 128-slot tiles).
  - Segment-sum runs on TensorE: per 128-edge tile a one-hot selection
    matrix S (built on VectorE via iota==dstrel) is matmul'd with the
    gathered messages, accumulating per-dst-block in PSUM.
  - PSUM is evicted with fused *sqrt_deg scaling into the f32 accumulator
    and the next layer's bf16 table staging.
  - Scoring (u/p/n dot products, softplus, ego L2 reg) is data-parallel
    over the 16384 batch (2048/core) + a final AllReduce.
"""

import sys
import numpy as np

sys.path.insert(0, "/opt/trn_rl_repo")

import ml_dtypes
import concourse.bacc as bacc
import concourse.bass as bass
import concourse.mybir as mybir
from concourse.bass_utils import run_bass_kernel_spmd
from concourse.tile import TileContext

# ---------------------------------------------------------------- config
N_NODES = 200000
N_EDGES = 1250000
D = 64
DC = 128            # table row padded to 128 cols bf16 = 256B
N_CORES = 8
BATCH = 16384
N_LAYERS = 3
LAM = 0.001
PART = N_NODES // N_CORES            # 25000
NB = (PART + 127) // 128             # 196 dst blocks / core
TROWS = NB * 128 + 16                # padded shard table rows (+16 zeroed pad)
GSIZE = 49                           # dst blocks per group (<=56: 7 PSUM banks)
NGROUP = (NB + GSIZE - 1) // GSIZE
BPC = BATCH // N_CORES               # 2048 batch items / core
BCOLS = BPC // 128                   # 16
F32 = mybir.dt.float32
BF16 = mybir.dt.bfloat16
I16 = mybir.dt.int16
I32 = mybir.dt.int32
AX = mybir.AxisListType
OP = mybir.AluOpType
AF = mybir.ActivationFunctionType

_MAX_WAIT_SPLIT_TYPES = ("InstDrain", "InstEventSemaphore", "InstHalt", "InstNoOp")


def _split_ctrl_waits(nc, max_waits=1):
    """walrus in this container accepts only one sync-wait on TPB_CTRL
    encodings; move extra waits onto single-wait nop carriers."""
    for fn in nc.m.functions:
        for blk in fn.blocks:
            insts = list(blk.instructions)
            out, changed = [], False
            for ins in insts:
                si = ins.sync_info
                nw = len(si.on_wait) if si and si.on_wait else 0
                if nw > max_waits and type(ins).__name__ in _MAX_WAIT_SPLIT_TYPES:
                    waits = list(si.on_wait)
                    for j, w in enumerate(waits[max_waits:]):
                        nop = mybir.InstNoOp(name=f"{ins.name}_ws{j}", ins=[], outs=[])
                        nop.engine = ins.engine
                        nop.sync_info = mybir.SyncInfo(on_wait=[w], on_update=[])
                        out.append(nop)
                    si.on_wait = waits[:max_waits]
                    changed = True
                out.append(ins)
            if changed:
                blk.instructions = out
    return nc


# ---------------------------------------------------------------- host prep
def _wrap_idx(arr):
    """[n] -> [128, n//16] int16 wrapped in 16 partitions, replicated 8x."""
    n = len(arr)
    w = arr.astype(np.int16).reshape(n // 16, 16).T
    return np.tile(w, (8, 1))


def _preprocess(emb, sqrt_deg, src, dst, users, pos, neg, n_users):
    src = np.asarray(src).astype(np.int64)
    dst = np.asarray(dst).astype(np.int64)
    emb = np.asarray(emb, dtype=np.float32)
    sd = np.asarray(sqrt_deg, dtype=np.float32)

    core = dst // PART
    dloc = dst - core * PART
    b = dloc // 128
    ps = src // PART
    sloc = (src - ps * PART).astype(np.int32)
    drel = (dloc - b * 128).astype(np.float32)

    bucket = (core * NB + b) * N_CORES + ps          # [E]
    order = np.argsort(bucket, kind="stable")
    counts = np.bincount(bucket, minlength=N_CORES * NB * N_CORES)
    counts3 = counts.reshape(N_CORES, NB, N_CORES)   # [core, b, ps]
    tmax = (np.max(counts3, axis=0) + 127) // 128    # [b, ps] common tiles
    tmax = np.maximum(tmax, 1)

    # common slot layout: g-major, ps, then blocks of g
    slot_off = np.zeros((NB, N_CORES), np.int64)     # slot offset of bucket
    call_meta = []                                   # (g, ps, n_idx, off)
    cur = 0
    tile_blocks = []                                 # per global tile: (g, ps, j)
    for g in range(NGROUP):
        blo, bhi = g * GSIZE, min((g + 1) * GSIZE, NB)
        for p in range(N_CORES):
            off0 = cur
            for bb in range(blo, bhi):
                slot_off[bb, p] = cur
                cur += int(tmax[bb, p]) * 128
                for _ in range(int(tmax[bb, p])):
                    tile_blocks.append((g, p, bb))
            call_meta.append((g, p, cur - off0, off0))
    tot_slots = cur
    assert tot_slots % 128 == 0
    t_total = tot_slots // 128
    for _, _, n, _ in call_meta:
        assert n <= 12800, f"gather call too big: {n}"

    # per-core streams
    starts = np.zeros(N_CORES * NB * N_CORES + 1, np.int64)
    np.cumsum(counts, out=starts[1:])
    skey = bucket[order]
    rank = np.arange(N_EDGES) - starts[skey]
    pos_in_stream = slot_off[(skey // N_CORES) % NB, skey % N_CORES] + rank
    core_sorted = skey // (NB * N_CORES)

    idx_streams = np.full((N_CORES, tot_slots), PART, np.int32)
    drel_streams = np.full((N_CORES, tot_slots), -1.0, np.float32)
    idx_streams[core_sorted, pos_in_stream] = sloc[order]
    drel_streams[core_sorted, pos_in_stream] = drel[order]

    # device tensors per core
    per_core = []
    sd2 = sd * sd
    users = np.asarray(users).astype(np.int64)
    posn = np.asarray(pos).astype(np.int64) + int(n_users)
    negn = np.asarray(neg).astype(np.int64) + int(n_users)

    def trow(node):
        return ((node // PART) * TROWS + node % PART).astype(np.int32)

    def bcol(node32):  # [2048] -> [128, 16] slot (p, j) = elem j*128+p
        return node32.reshape(BCOLS, 128).T.copy()

    iota = np.tile(np.arange(128, dtype=np.float32), (128, 1)).astype(ml_dtypes.bfloat16)
    ones = np.ones((128, 1), np.float32)

    for c in range(N_CORES):
        lo = c * PART
        e = emb[lo:lo + PART]                        # [25000, 64]
        epb = np.zeros((128, NB * D), np.float32)
        ex = np.zeros((NB * 128, D), np.float32)
        ex[:PART] = e
        epb[:] = ex.reshape(NB, 128, D).transpose(1, 0, 2).reshape(128, NB * D)
        sdx = np.zeros(NB * 128, np.float32)
        sdx[:PART] = sd[lo:lo + PART]
        sd_pb = sdx.reshape(NB, 128).T.copy()
        sd2x = np.zeros(NB * 128, np.float32)
        sd2x[:PART] = sd2[lo:lo + PART]
        sd2_pb = sd2x.reshape(NB, 128).T.copy()

        gidx = np.hstack([_wrap_idx(idx_streams[c, off:off + n])
                          for (_, _, n, off) in call_meta])
        drel_t = drel_streams[c].reshape(t_total, 128).T.copy()

        cnt = np.bincount(users, minlength=N_NODES).astype(np.float32)
        cnt += np.bincount(posn, minlength=N_NODES)
        cnt += np.bincount(negn, minlength=N_NODES)
        cx = np.zeros(NB * 128, np.float32)
        cx[:PART] = cnt[lo:lo + PART]
        cnt_pb = cx.reshape(NB, 128).T.copy()

        sl = slice(c * BPC, (c + 1) * BPC)
        per_core.append({
            "cnt_pb": cnt_pb,
            "emb_pb": epb,
            "sd_pb": sd_pb,
            "sd2_pb": sd2_pb,
            "gidx": gidx,
            "dstrel": drel_t,
            "iota": iota,
            "ones": ones,
            "u_tr": bcol(trow(users[sl])), "p_tr": bcol(trow(posn[sl])),
            "n_tr": bcol(trow(negn[sl])),
        })

    meta = {
        "call_meta": call_meta,
        "tile_blocks": tile_blocks,
        "t_total": t_total,
        "gidx16": per_core[0]["gidx"].shape[1],
    }
    return per_core, meta


# ---------------------------------------------------------------- builder
def _build(meta):
    call_meta = meta["call_meta"]
    tile_blocks = meta["tile_blocks"]
    t_total = meta["t_total"]
    gidx16 = meta["gidx16"]

    # per-block ordered tile list (global tile ids)
    blk_tiles = [[] for _ in range(NB)]
    for t, (g, p, bb) in enumerate(tile_blocks):
        blk_tiles[bb].append(t)
    # chunk (g,ps) -> (tile range, idx16 range)
    chunk_of = {}
    t0 = 0
    off16 = 0
    for (g, p, n, off) in call_meta:
        nt = n // 128
        chunk_of[(g, p)] = (t0, nt, off16, n)
        t0 += nt
        off16 += n // 16
    maxt = max(nt for (_, nt, _, _) in chunk_of.values())

    nc = bacc.Bacc("TRN2", num_swdge_queues=4)
    cnt_pb = nc.dram_tensor("cnt_pb", [128, NB], F32, kind="ExternalInput")
    emb_pb = nc.dram_tensor("emb_pb", [128, NB * D], F32, kind="ExternalInput")
    sd_pb = nc.dram_tensor("sd_pb", [128, NB], F32, kind="ExternalInput")
    sd2_pb = nc.dram_tensor("sd2_pb", [128, NB], F32, kind="ExternalInput")
    gidx = nc.dram_tensor("gidx", [128, gidx16], I16, kind="ExternalInput")
    dstrel = nc.dram_tensor("dstrel", [128, t_total], F32, kind="ExternalInput")
    iota = nc.dram_tensor("iota", [128, 128], BF16, kind="ExternalInput")
    ones = nc.dram_tensor("ones", [128, 1], F32, kind="ExternalInput")
    btens = {nm: nc.dram_tensor(nm, [128, BCOLS], I32, kind="ExternalInput")
             for nm in ("u_tr", "p_tr", "n_tr")}
    loss_o = nc.dram_tensor("loss", [1, 2], F32, kind="ExternalOutput")

    bounce = nc.dram_tensor("bounce", [TROWS, DC], BF16)
    table = nc.dram_tensor("table", [N_CORES * TROWS, DC], BF16)
    red_i = nc.dram_tensor("red_i", [1, 2], F32)
    red_o = nc.dram_tensor("red_o", [1, 2], F32)

    groups = [(g * GSIZE, min((g + 1) * GSIZE, NB)) for g in range(NGROUP)]
    rg = [list(range(N_CORES))]

    with TileContext(nc) as tc:
        with (tc.tile_pool(name="per", bufs=1) as per,
              tc.tile_pool(name="ring", bufs=2) as ring,
              tc.tile_pool(name="psr", bufs=1, space="PSUM") as psr):
            # resident tiles
            acc = per.tile([128, NB * D], F32, name="acc")
            sd_t = per.tile([128, NB], F32, name="sd_t")
            sd2_t = per.tile([128, NB], F32, name="sd2_t")
            gidx_t = per.tile([128, gidx16], I16, name="gidx_t")
            drel_t = per.tile([128, t_total], F32, name="drel_t")
            iota_t = per.tile([128, 128], BF16, name="iota_t")
            ones_t = per.tile([128, 1], F32, name="ones_t")
            nc.sync.dma_start(acc[:], emb_pb[:])
            nc.sync.dma_start(sd_t[:], sd_pb[:])
            nc.sync.dma_start(sd2_t[:], sd2_pb[:])
            nc.sync.dma_start(gidx_t[:], gidx[:])
            nc.sync.dma_start(drel_t[:], dstrel[:])
            nc.sync.dma_start(iota_t[:], iota[:])
            nc.sync.dma_start(ones_t[:], ones[:])
            zpad = per.tile([16, DC], BF16, name="zpad")
            nc.vector.memset(zpad[:], 0.0)
            nc.sync.dma_start(bounce[NB * 128:TROWS, :], zpad[:])
            cnt_t = per.tile([128, NB], F32, name="cnt_t")
            nc.sync.dma_start(cnt_t[:], cnt_pb[:])
            regcol = per.tile([128, NB, 1], F32, name="regcol")
            regp = per.tile([128, 1], F32, name="regp")
            for g0 in range(0, NB, GSIZE):
                g1 = min(g0 + GSIZE, NB)
                rsq = ring.tile([128, GSIZE * D], F32, tag="rsq", name=f"rsq_{g0}")
                nc.vector.tensor_tensor(
                    out=rsq[:, 0:(g1 - g0) * D],
                    in0=acc[:, g0 * D:g1 * D], in1=acc[:, g0 * D:g1 * D],
                    op=OP.mult)
                nc.vector.reduce_sum(
                    regcol[:, g0:g1, :],
                    rsq[:, 0:(g1 - g0) * D].rearrange("p (b d) -> p b d", d=D),
                    axis=AX.X)
            nc.vector.tensor_tensor(
                out=regcol[:, :, 0], in0=regcol[:, :, 0], in1=cnt_t[:],
                op=OP.mult)
            nc.vector.reduce_sum(regp[:], regcol[:], axis=AX.XY)

            def stage_group(g, blo, bhi, src_fn, li):
                """write bf16 staged table rows for group's blocks, DMA out."""
                n = bhi - blo
                st = ring.tile([128, GSIZE * DC], BF16, tag="stage",
                               name=f"st_{li}_{g}")
                for j in range(n):
                    src_fn(j, st[:, j * DC: j * DC + D])
                dstv = bounce[blo * 128:(blo + n) * 128, :].rearrange(
                    "(j p) d -> p j d", p=128)[:, :, 0:D]
                nc.sync.dma_start(dstv, st[:, 0:n * DC].rearrange(
                    "p (j d) -> p j d", d=DC)[:, :, 0:D])

            # ---- initial table: emb * sd
            for g, (blo, bhi) in enumerate(groups):
                def src0(j, out, blo=blo):
                    bb = blo + j
                    nc.vector.tensor_scalar(
                        out=out, in0=acc[:, bb * D:(bb + 1) * D],
                        scalar1=sd_t[:, bb:bb + 1], scalar2=None, op0=OP.mult)
                stage_group(g, blo, bhi, src0, "i")
            nc.gpsimd.collective_compute(
                "AllGather", OP.bypass, replica_groups=rg,
                ins=[bounce[:].opt()], outs=[table[:].opt()])

            # ---- layers
            for li in range(N_LAYERS):
                last = li == N_LAYERS - 1
                for g, (blo, bhi) in enumerate(groups):
                    nblk = bhi - blo
                    nbank = (nblk + 7) // 8
                    banks = [psr.tile([128, 512], F32, tag=f"pb{k}",
                                      name=f"pb_{li}_{g}_{k}")
                             for k in range(nbank)]

                    def pslice(bb):
                        j = bb - blo
                        return banks[j // 8][:, (j % 8) * 64:(j % 8) * 64 + 64]

                    chunks = []
                    for p in range(N_CORES):
                        ct0, nt, o16, n = chunk_of[(g, p)]
                        m = ring.tile([128, maxt * DC], BF16, tag="msgs",
                                      name=f"m_{li}_{g}_{p}")
                        s = ring.tile([128, maxt * 128], BF16, tag="sgen",
                                      name=f"s_{li}_{g}_{p}")
                        nc.gpsimd.dma_gather(
                            m[:, 0:nt * DC].rearrange("p (t d) -> p t d", d=DC),
                            table[p * TROWS:(p + 1) * TROWS, :],
                            gidx_t[:, o16:o16 + n // 16],
                            n, n, DC, single_packet=False,
                            queue_num=p % 4)
                        chunks.append((ct0, nt, m, s))
                    for p in range(N_CORES):
                        ct0, nt, m, s = chunks[p]
                        for tl in range(nt):
                            t = ct0 + tl
                            bb = tile_blocks[t][2]
                            sv = s[:, tl * 128:(tl + 1) * 128]
                            nc.vector.tensor_scalar(
                                out=sv, in0=iota_t[:],
                                scalar1=drel_t[:, t:t + 1], scalar2=None,
                                op0=OP.is_equal)
                            tlist = blk_tiles[bb]
                            nc.tensor.matmul(
                                pslice(bb), lhsT=sv,
                                rhs=m[:, tl * DC:tl * DC + D],
                                start=(t == tlist[0]), stop=(t == tlist[-1]))
                    # evictions
                    for bb in range(blo, bhi):
                        tmp = ring.tile([128, 64], F32, tag="ev",
                                        name=f"ev_{li}_{g}_{bb}")
                        nc.vector.tensor_scalar(
                            out=tmp[:], in0=pslice(bb),
                            scalar1=sd_t[:, bb:bb + 1], scalar2=None,
                            op0=OP.mult)
                        nc.vector.tensor_tensor(
                            out=acc[:, bb * D:(bb + 1) * D],
                            in0=acc[:, bb * D:(bb + 1) * D], in1=tmp[:],
                            op=OP.add)
                    if not last:
                        def srcn(j, out, blo=blo):
                            bb = blo + j
                            nc.vector.tensor_scalar(
                                out=out, in0=pslice(bb),
                                scalar1=sd2_t[:, bb:bb + 1], scalar2=None,
                                op0=OP.mult)
                        stage_group(g, blo, bhi, srcn, li)
                if not last:
                    nc.gpsimd.collective_compute(
                        "AllGather", OP.bypass, replica_groups=rg,
                        ins=[bounce[:].opt()], outs=[table[:].opt()])

            # ---- prop table: acc * 0.25
            for g, (blo, bhi) in enumerate(groups):
                def srcp(j, out, blo=blo):
                    bb = blo + j
                    nc.vector.tensor_scalar_mul(
                        out, acc[:, bb * D:(bb + 1) * D], 0.25)
                stage_group(g, blo, bhi, srcp, "p")
            nc.gpsimd.collective_compute(
                "AllGather", OP.bypass, replica_groups=rg,
                ins=[bounce[:].opt()], outs=[table[:].opt()])

            # ---- scoring
            bt = {nm: per.tile([128, BCOLS], I32, name=f"bt_{nm}")
                  for nm in btens}
            for nm in btens:
                nc.sync.dma_start(bt[nm][:], btens[nm][:])
            prop = {}
            for nm in ("u_tr", "p_tr", "n_tr"):
                pt = per.tile([128, BCOLS, DC], BF16, name=f"prop_{nm}")
                for j in range(BCOLS):
                    nc.gpsimd.indirect_dma_start(
                        out=pt[:, j, :], out_offset=None, in_=table[:],
                        in_offset=bass.IndirectOffsetOnAxis(
                            ap=bt[nm][:, j:j + 1], axis=0))
                prop[nm] = pt
            def dots(a, b, name):
                m = per.tile([128, BCOLS, D], F32, name=f"dm_{name}")
                nc.vector.tensor_tensor(out=m[:], in0=a[:, :, 0:D],
                                        in1=b[:, :, 0:D], op=OP.mult)
                r = per.tile([128, BCOLS, 1], F32, name=f"dr_{name}")
                nc.vector.reduce_sum(r[:], m[:], axis=AX.X)
                return r

            ps_s = dots(prop["u_tr"], prop["p_tr"], "pos")
            ns_s = dots(prop["u_tr"], prop["n_tr"], "neg")
            diff = per.tile([128, BCOLS, 1], F32, name="diff")
            nc.vector.tensor_tensor(out=diff[:], in0=ns_s[:], in1=ps_s[:],
                                    op=OP.subtract)
            expd = per.tile([128, BCOLS, 1], F32, name="expd")
            nc.scalar.activation(expd[:], diff[:], AF.Exp)
            sp = per.tile([128, BCOLS, 1], F32, name="sp")
            nc.scalar.activation(sp[:], expd[:], AF.Ln, bias=1.0)

            two = per.tile([128, 2], F32, name="two")
            nc.vector.reduce_sum(two[:, 0:1], sp[:], axis=AX.XY)
            nc.vector.tensor_copy(two[:, 1:2], regp[:])
            rp = psr.tile([128, 2], F32, tag="rps", name="rps")
            nc.tensor.matmul(rp[0:1, 0:2], lhsT=ones_t[:], rhs=two[:],
                             start=True, stop=True)
            redt = per.tile([1, 2], F32, name="redt")
            nc.vector.tensor_copy(redt[:], rp[0:1, 0:2])
            nc.sync.dma_start(red_i[:], redt[:])
            nc.gpsimd.collective_compute(
                "AllReduce", OP.add, replica_groups=rg,
                ins=[red_i[:].opt()], outs=[red_o[:].opt()])
            fin = per.tile([1, 2], F32, name="fin")
            nc.sync.dma_start(fin[:], red_o[:])
            nc.sync.dma_start(loss_o[:], fin[:])
    nc.compile()
    _split_ctrl_waits(nc)
    return nc


def kernel(**inputs) -> np.ndarray:
    per_core, meta = _preprocess(
        inputs["emb"], inputs["sqrt_deg"], inputs["src"], inputs["dst"],
        inputs["users"], inputs["pos"], inputs["neg"], inputs["n_users"])
    nc = _build(meta)
    res = None
    for attempt in range(3):
        try:
            res = run_bass_kernel_spmd(nc, per_core, core_ids=list(range(N_CORES)))
            break
        except Exception:
            # transient NRT exec-unit flake: device recovers on re-dispatch
            if attempt == 2:
                raise
            import time as _t
            _t.sleep(5)
    two = res.results[0]["loss"].reshape(2).astype(np.float64)
    loss = two[0] / BATCH + LAM * 0.5 * two[1] / BATCH
    return np.asarray(loss, dtype=np.float32).reshape(())
